# revision 10
# baseline (speedup 1.0000x reference)
"""Routed MoE classifier head for Trainium2 (8 NeuronCores, SPMD).

The reference computes all 8 experts densely and selects; here each sample is
routed to exactly one expert.  On the host we gather samples by expert
(expert e -> core e), pad to a common S, and pre-transpose x so the
contraction dim D lands on SBUF partitions.  Each core runs a dense 2-layer
MLP (768 -> relu 384 -> 8) over its expert's samples:

  layer 1:  h^T = relu(W1^T x^T + b1)   as matmul(psum, lhsT=W1 [128,128],
            rhs=xT [128,n]) accumulated over 6 d-blocks per h-block
  layer 2:  y^T = W2^T h^T + b2

Matmul operands use float32r (fp32 bits, 1 column/cycle streaming — 4x the
fp32 paired-pass rate — with ~11-mantissa-bit operand rounding); PSUM
accumulation stays fp32.  Output y^T [8, S] is scattered back on the host.
"""

import ml_dtypes
import numpy as np

import concourse.bass as bass
import concourse.mybir as mybir
from concourse.tile import TileContext
from concourse.bass_utils import run_bass_kernel_spmd

P = 128
D = 768
H = 384
C = 8
E = 8
NCORES = 8
DBLK = D // P  # 6
HBLK = H // P  # 3
CHUNK = 512  # compute chunk (one PSUM bank of fp32)
XGRAN = 1536  # x DMA granularity (samples per load; multiple of CHUNK)
YGRAN = 2048  # y DMA granularity (samples per store)

MM_DTYPE = "bf16"

_program_cache = {}
last_results = None  # BassKernelResults of the most recent run (for test harness)


def _split_excess_waits(nc, max_waits=1):
    """The walrus build in this container only encodes one sem-wait per
    instruction; hoist extra waits onto NOPs inserted just before."""
    for blk in nc.main_func.blocks:
        insts = blk.instructions
        i = 0
        while i < len(insts):
            inst = insts[i]
            si = getattr(inst, "sync_info", None)
            if si is not None and si.on_wait and len(si.on_wait) > max_waits:
                waits = list(si.on_wait)
                extra, keep = waits[:-max_waits], waits[-max_waits:]
                nops = []
                for j in range(0, len(extra), max_waits):
                    nops.append(
                        mybir.InstNoOp(
                            name=f"{inst.name}-wsplit{j}",
                            engine=inst.engine,
                            bass_nofuse=True,
                            sync_info=mybir.SyncInfo(
                                on_wait=extra[j : j + max_waits], on_update=[]
                            ),
                        )
                    )
                inst.sync_info = mybir.SyncInfo(on_wait=keep, on_update=si.on_update)
                for k, nop in enumerate(nops):
                    nc.register_instruction(nop, overwrite=True)
                    insts.insert(i + k, nop)
                i += len(nops)
            i += 1
    return nc


def _spans2(total, lead, gran):
    """[(off, n), ...] covering `total`: leading spans from `lead`, then
    `gran`-sized spans (last one smaller)."""
    spans = []
    off = 0
    k = 0
    while off < total:
        n = min(lead[k] if k < len(lead) else gran, total - off)
        spans.append((off, n))
        off += n
        k += 1
    return spans


def _build_program(S):
    f32 = mybir.dt.float32
    fmm = {"f32r": mybir.dt.float32r, "bf16": mybir.dt.bfloat16}.get(MM_DTYPE, f32)
    relu = mybir.ActivationFunctionType.Relu
    add = mybir.AluOpType.add

    nc = bass.Bass(enable_partition_id=False)
    xt = nc.dram_tensor("xt", [P, DBLK, S], fmm, kind="ExternalInput")
    # w1 (6*384 cols) and w2 (3*8 cols) packed on the same 128 partitions
    wt = nc.dram_tensor("wt", [P, DBLK * H + HBLK * C], fmm, kind="ExternalInput")
    # b1 (3 cols, per h-block) and b2 (1 col, rows 0..7) packed
    bt = nc.dram_tensor("bt", [P, HBLK + 1], f32, kind="ExternalInput")
    yt = nc.dram_tensor("yt", [C, S], f32, kind="ExternalOutput")

    x_spans = _spans2(S, [CHUNK, CHUNK, CHUNK], XGRAN)

    with TileContext(nc) as tc:
        with (
            tc.tile_pool(name="const", bufs=1) as cpool,
            tc.tile_pool(name="xin", bufs=3) as xpool,
            tc.tile_pool(name="hbuf", bufs=3) as hpool,
            tc.tile_pool(name="yout", bufs=2) as ypool,
            tc.tile_pool(name="psum1", bufs=6, space="PSUM") as pp1,
            tc.tile_pool(name="psum2", bufs=2, space="PSUM") as pp2,
        ):
            # Startup critical path: the first matmul (db=0) needs only the
            # db=0 pieces of W1 and x-span0, so interleave those on the sync
            # queue; span 1 rides the scalar queue in parallel.  b/W2/ACT
            # warmup ride the scalar queue behind span 1 (plenty of slack:
            # first use is the layer-2 epilogue of chunk 1).
            b_t = cpool.tile([P, HBLK + 1], f32)
            w_t = cpool.tile([P, DBLK * H + HBLK * C], fmm)
            span_tiles = {}
            nxt = 0  # next span index to issue

            def load_x_span(engines):
                nonlocal nxt
                si = nxt
                nxt += 1
                off, n = x_spans[si]
                x_t = xpool.tile([P, DBLK, XGRAN], fmm, name="x_t")
                for db in range(DBLK):
                    engines[db % len(engines)].dma_start(
                        x_t[:, db, :n], xt[:, db, off : off + n]
                    )
                span_tiles[si] = x_t

            off0, n0 = x_spans[0]
            x0 = xpool.tile([P, DBLK, XGRAN], fmm, name="x_t")
            for db in range(DBLK):
                nc.sync.dma_start(
                    w_t[:, db * H : (db + 1) * H], wt[:, db * H : (db + 1) * H]
                )
                nc.sync.dma_start(x0[:, db, :n0], xt[:, db, off0 : off0 + n0])
            span_tiles[0] = x0
            nxt = 1
            if len(x_spans) > 1:
                load_x_span([nc.scalar])  # span 1
            nc.scalar.dma_start(b_t[:], bt[:])
            nc.scalar.dma_start(w_t[:, DBLK * H :], wt[:, DBLK * H :])
            if len(x_spans) > 2:
                load_x_span([nc.sync])  # span 2 behind W1/span0 on sync

            # Warm the ACT table during the startup DMA window so the
            # first real relu doesn't pay the ~1.5us table load.
            warm = cpool.tile([P, 1], f32)
            nc.any.memset(warm[:], 0.0)
            nc.scalar.activation(warm[:], warm[:], relu, bias=0.0)

            y_tile = None  # current [C, YGRAN] output staging tile
            y_base = 0

            def emit_l2(pend):
                # layer 2 for an already-relu'd chunk: y^T = W2^T h^T + b2
                nonlocal y_tile, y_base
                h_t, off, n = pend
                ps2 = pp2.tile([C, CHUNK], f32, name="ps2")
                for hb in range(HBLK):
                    nc.tensor.matmul(
                        ps2[:, :n],
                        w_t[:, DBLK * H + hb * C : DBLK * H + (hb + 1) * C],
                        h_t[:, hb, :n],
                        start=(hb == 0),
                        stop=(hb == HBLK - 1),
                    )
                if y_tile is None:
                    y_tile = ypool.tile([C, YGRAN], f32, name="y_t")
                    y_base = off
                lo = off - y_base
                nc.vector.tensor_scalar(
                    y_tile[:, lo : lo + n],
                    ps2[:, :n],
                    scalar1=b_t[:C, HBLK : HBLK + 1],
                    scalar2=None,
                    op0=add,
                )
                if lo + n + CHUNK > YGRAN or off + n >= S:
                    nc.sync.dma_start(yt[:, y_base : y_base + lo + n], y_tile[:, : lo + n])
                    y_tile = None

            chunks = []  # (span_idx, global_off, local_off, n)
            for si, (soff, sn) in enumerate(x_spans):
                for o in range(0, sn, CHUNK):
                    chunks.append((si, soff + o, o, min(CHUNK, sn - o)))

            # Chunk 0 runs db-outer so each arriving x d-block slice feeds
            # all three h-block accumulators immediately (DMA drip-feed).
            # Steady-state chunks run hb-outer/db-inner so relu(hb) overlaps
            # the next h-block's matmul stream within the same chunk, and
            # layer-2 of chunk k-1 (emitted at the end of chunk k) has a
            # full chunk of slack behind relu(k-1, h2).
            pending = None
            for ci, (si, goff, o, n) in enumerate(chunks):
                x_t = span_tiles[si]
                if o == 0 and ci > 0 and nxt < len(x_spans):
                    load_x_span([nc.sync, nc.scalar] if ci % 2 else [nc.scalar, nc.sync])
                h_t = hpool.tile([P, HBLK, CHUNK], fmm, name="h_t")
                if ci == 0:
                    pss = [pp1.tile([P, CHUNK], f32, name="ps") for _ in range(HBLK)]
                    for db in range(DBLK):
                        for hb in range(HBLK):
                            nc.tensor.matmul(
                                pss[hb][:, :n],
                                w_t[:, db * H + hb * P : db * H + (hb + 1) * P],
                                x_t[:, db, o : o + n],
                                start=(db == 0),
                                stop=(db == DBLK - 1),
                            )
                    for hb in range(HBLK):
                        nc.scalar.activation(
                            h_t[:, hb, :n], pss[hb][:, :n], relu,
                            bias=b_t[:, hb : hb + 1],
                        )
                else:
                    for hb in range(HBLK):
                        ps = pp1.tile([P, CHUNK], f32, name="ps")
                        for db in range(DBLK):
                            nc.tensor.matmul(
                                ps[:, :n],
                                w_t[:, db * H + hb * P : db * H + (hb + 1) * P],
                                x_t[:, db, o : o + n],
                                start=(db == 0),
                                stop=(db == DBLK - 1),
                            )
                        nc.scalar.activation(
                            h_t[:, hb, :n], ps[:, :n], relu,
                            bias=b_t[:, hb : hb + 1],
                        )
                if pending is not None:
                    emit_l2(pending)
                if o + n >= x_spans[si][1]:
                    span_tiles.pop(si, None)
                pending = (h_t, goff, n)
            emit_l2(pending)

    return _split_excess_waits(nc)


def kernel(x, W1, b1, W2, b2, question_types):
    global last_results
    x = np.ascontiguousarray(np.asarray(x, dtype=np.float32))
    W1 = np.asarray(W1, dtype=np.float32)
    b1 = np.asarray(b1, dtype=np.float32)
    W2 = np.asarray(W2, dtype=np.float32)
    b2 = np.asarray(b2, dtype=np.float32)
    qt = np.asarray(question_types)
    N = x.shape[0]

    idx = [np.nonzero(qt == e)[0] for e in range(E)]
    counts = [len(i) for i in idx]
    S = max(int(np.ceil(max(counts) / 16) * 16), 2 * CHUNK)

    nc = _program_cache.get(S)
    if nc is None:
        nc = _build_program(S)
        _program_cache[S] = nc

    mmnp = {"f32r": np.float32, "bf16": ml_dtypes.bfloat16}.get(MM_DTYPE, np.float32)
    in_maps = []
    for e in range(E):
        cnt = counts[e]
        xp = np.zeros((S, D), mmnp)
        xp[:cnt] = x[idx[e]].astype(mmnp)
        xt = np.ascontiguousarray(xp.T.reshape(DBLK, P, S).transpose(1, 0, 2))
        w1t = W1[e].reshape(DBLK, P, H).transpose(1, 0, 2).reshape(P, DBLK * H)
        w2t = W2[e].reshape(HBLK, P, C).transpose(1, 0, 2).reshape(P, HBLK * C)
        wt = np.ascontiguousarray(np.concatenate([w1t, w2t], axis=1)).astype(mmnp)
        bt = np.zeros((P, HBLK + 1), np.float32)
        bt[:, :HBLK] = b1[e].reshape(HBLK, P).T
        bt[:C, HBLK] = b2[e]
        in_maps.append({"xt": xt, "wt": wt, "bt": bt})

    r = run_bass_kernel_spmd(nc, in_maps, list(range(NCORES)))
    last_results = r

    out = np.zeros((N, C), np.float32)
    for e in range(E):
        out[idx[e]] = r.results[e]["yt"][:, : counts[e]].T
    return out



# revision 12
# speedup vs baseline: 1.0150x; 1.0150x over previous
"""Routed MoE classifier head for Trainium2 (8 NeuronCores, SPMD).

The reference computes all 8 experts densely and selects; here each sample is
routed to exactly one expert.  On the host we gather samples by expert
(expert e -> core e), pad to a common S, and pre-transpose x so the
contraction dim D lands on SBUF partitions.  Each core runs a dense 2-layer
MLP (768 -> relu 384 -> 8) over its expert's samples:

  layer 1:  h^T = relu(W1^T x^T + b1)   as matmul(psum, lhsT=W1 [128,128],
            rhs=xT [128,n]) accumulated over 6 d-blocks per h-block
  layer 2:  y^T = W2^T h^T + b2

Matmul operands use float32r (fp32 bits, 1 column/cycle streaming — 4x the
fp32 paired-pass rate — with ~11-mantissa-bit operand rounding); PSUM
accumulation stays fp32.  Output y^T [8, S] is scattered back on the host.
"""

import ml_dtypes
import numpy as np

import concourse.bass as bass
import concourse.mybir as mybir
from concourse.tile import TileContext
from concourse.bass_utils import run_bass_kernel_spmd

P = 128
D = 768
H = 384
C = 8
E = 8
NCORES = 8
DBLK = D // P  # 6
HBLK = H // P  # 3
CHUNK = 512  # compute chunk (one PSUM bank of fp32)
XGRAN = 1536  # x DMA granularity (samples per load; multiple of CHUNK)
YGRAN = 2048  # y DMA granularity (samples per store)

MM_DTYPE = "bf16"

_program_cache = {}
last_results = None  # BassKernelResults of the most recent run (for test harness)


def _split_excess_waits(nc, max_waits=1):
    """The walrus build in this container only encodes one sem-wait per
    instruction; hoist extra waits onto NOPs inserted just before."""
    for blk in nc.main_func.blocks:
        insts = blk.instructions
        i = 0
        while i < len(insts):
            inst = insts[i]
            si = getattr(inst, "sync_info", None)
            if si is not None and si.on_wait and len(si.on_wait) > max_waits:
                waits = list(si.on_wait)
                extra, keep = waits[:-max_waits], waits[-max_waits:]
                nops = []
                for j in range(0, len(extra), max_waits):
                    nops.append(
                        mybir.InstNoOp(
                            name=f"{inst.name}-wsplit{j}",
                            engine=inst.engine,
                            bass_nofuse=True,
                            sync_info=mybir.SyncInfo(
                                on_wait=extra[j : j + max_waits], on_update=[]
                            ),
                        )
                    )
                inst.sync_info = mybir.SyncInfo(on_wait=keep, on_update=si.on_update)
                for k, nop in enumerate(nops):
                    nc.register_instruction(nop, overwrite=True)
                    insts.insert(i + k, nop)
                i += len(nops)
            i += 1
    return nc


def _spans2(total, lead, gran):
    """[(off, n), ...] covering `total`: leading spans from `lead`, then
    `gran`-sized spans (last one smaller)."""
    spans = []
    off = 0
    k = 0
    while off < total:
        n = min(lead[k] if k < len(lead) else gran, total - off)
        spans.append((off, n))
        off += n
        k += 1
    return spans


def _build_program(S):
    f32 = mybir.dt.float32
    fmm = {"f32r": mybir.dt.float32r, "bf16": mybir.dt.bfloat16}.get(MM_DTYPE, f32)
    relu = mybir.ActivationFunctionType.Relu
    add = mybir.AluOpType.add

    nc = bass.Bass(enable_partition_id=False)
    xt = nc.dram_tensor("xt", [P, DBLK, S], fmm, kind="ExternalInput")
    # w1 (6*384 cols) and w2 (3*8 cols) packed on the same 128 partitions
    wt = nc.dram_tensor("wt", [P, DBLK * H + HBLK * C], fmm, kind="ExternalInput")
    # b1 (3 cols, per h-block) and b2 (1 col, rows 0..7) packed
    bt = nc.dram_tensor("bt", [P, HBLK + 1], f32, kind="ExternalInput")
    yt = nc.dram_tensor("yt", [C, S], f32, kind="ExternalOutput")

    x_spans = _spans2(S, [CHUNK, CHUNK, CHUNK], XGRAN)

    with TileContext(nc) as tc:
        with (
            tc.tile_pool(name="const", bufs=1) as cpool,
            tc.tile_pool(name="xin", bufs=3) as xpool,
            tc.tile_pool(name="hbuf", bufs=3) as hpool,
            tc.tile_pool(name="yout", bufs=2) as ypool,
            tc.tile_pool(name="psum1", bufs=6, space="PSUM") as pp1,
            tc.tile_pool(name="psum2", bufs=2, space="PSUM") as pp2,
        ):
            # DMA descriptor generation costs the issuing engine ~0.6us of
            # queue time per dma_start, so steady-state x spans are ONE
            # descriptor each on the sync queue (which otherwise only does
            # y stores); the scalar queue stays clean for relus.  Startup:
            # x span 0 drips per-d-block on sync (chunk 0 is db-outer), W1
            # rides scalar as [db=0 | rest] so the first matmul only waits
            # for its own 96KB.
            b_t = cpool.tile([P, HBLK + 1], f32)
            w_t = cpool.tile([P, DBLK * H + HBLK * C], fmm)
            span_tiles = {}
            nxt = 0  # next span index to issue

            def load_x_span(eng):
                nonlocal nxt
                si = nxt
                nxt += 1
                off, n = x_spans[si]
                x_t = xpool.tile([P, DBLK, XGRAN], fmm, name="x_t")
                eng.dma_start(x_t[:, :, :n], xt[:, :, off : off + n])
                span_tiles[si] = x_t

            off0, n0 = x_spans[0]
            x0 = xpool.tile([P, DBLK, XGRAN], fmm, name="x_t")
            for db in range(DBLK):
                nc.sync.dma_start(x0[:, db, :n0], xt[:, db, off0 : off0 + n0])
            span_tiles[0] = x0
            nxt = 1
            nc.scalar.dma_start(w_t[:, :H], wt[:, :H])  # W1 db=0 blocks
            nc.scalar.dma_start(w_t[:, H:], wt[:, H:])  # rest of W1 + W2
            nc.scalar.dma_start(b_t[:], bt[:])
            if len(x_spans) > 1:
                load_x_span(nc.scalar)  # span 1
            if len(x_spans) > 2:
                load_x_span(nc.sync)  # span 2

            # Warm the ACT table during the startup DMA window so the
            # first real relu doesn't pay the ~1.5us table load.
            warm = cpool.tile([P, 1], f32)
            nc.any.memset(warm[:], 0.0)
            nc.scalar.activation(warm[:], warm[:], relu, bias=0.0)

            y_tile = None  # current [C, YGRAN] output staging tile
            y_base = 0

            def emit_l2(pend):
                # layer 2 for an already-relu'd chunk: y^T = W2^T h^T + b2
                nonlocal y_tile, y_base
                h_t, off, n = pend
                ps2 = pp2.tile([C, CHUNK], f32, name="ps2")
                for hb in range(HBLK):
                    nc.tensor.matmul(
                        ps2[:, :n],
                        w_t[:, DBLK * H + hb * C : DBLK * H + (hb + 1) * C],
                        h_t[:, hb, :n],
                        start=(hb == 0),
                        stop=(hb == HBLK - 1),
                    )
                if y_tile is None:
                    y_tile = ypool.tile([C, YGRAN], f32, name="y_t")
                    y_base = off
                lo = off - y_base
                nc.vector.tensor_scalar(
                    y_tile[:, lo : lo + n],
                    ps2[:, :n],
                    scalar1=b_t[:C, HBLK : HBLK + 1],
                    scalar2=None,
                    op0=add,
                )
                if lo + n + CHUNK > YGRAN or off + n >= S:
                    nc.sync.dma_start(yt[:, y_base : y_base + lo + n], y_tile[:, : lo + n])
                    y_tile = None

            chunks = []  # (span_idx, global_off, local_off, n)
            for si, (soff, sn) in enumerate(x_spans):
                for o in range(0, sn, CHUNK):
                    chunks.append((si, soff + o, o, min(CHUNK, sn - o)))

            # Chunk 0 runs db-outer so each arriving x d-block slice feeds
            # all three h-block accumulators immediately (DMA drip-feed).
            # Steady-state chunks run hb-outer/db-inner so relu(hb) overlaps
            # the next h-block's matmul stream within the same chunk, and
            # layer-2 of chunk k-1 (emitted at the end of chunk k) has a
            # full chunk of slack behind relu(k-1, h2).
            pending = None
            for ci, (si, goff, o, n) in enumerate(chunks):
                x_t = span_tiles[si]
                if o == 0 and ci > 0 and nxt < len(x_spans):
                    load_x_span(nc.sync)
                h_t = hpool.tile([P, HBLK, CHUNK], fmm, name="h_t")
                if ci == 0:
                    pss = [pp1.tile([P, CHUNK], f32, name="ps") for _ in range(HBLK)]
                    for db in range(DBLK):
                        for hb in range(HBLK):
                            nc.tensor.matmul(
                                pss[hb][:, :n],
                                w_t[:, db * H + hb * P : db * H + (hb + 1) * P],
                                x_t[:, db, o : o + n],
                                start=(db == 0),
                                stop=(db == DBLK - 1),
                            )
                    for hb in range(HBLK):
                        nc.scalar.activation(
                            h_t[:, hb, :n], pss[hb][:, :n], relu,
                            bias=b_t[:, hb : hb + 1],
                        )
                else:
                    for hb in range(HBLK):
                        ps = pp1.tile([P, CHUNK], f32, name="ps")
                        for db in range(DBLK):
                            nc.tensor.matmul(
                                ps[:, :n],
                                w_t[:, db * H + hb * P : db * H + (hb + 1) * P],
                                x_t[:, db, o : o + n],
                                start=(db == 0),
                                stop=(db == DBLK - 1),
                            )
                        nc.scalar.activation(
                            h_t[:, hb, :n], ps[:, :n], relu,
                            bias=b_t[:, hb : hb + 1],
                        )
                if pending is not None:
                    emit_l2(pending)
                if o + n >= x_spans[si][1]:
                    span_tiles.pop(si, None)
                pending = (h_t, goff, n)
            emit_l2(pending)

    return _split_excess_waits(nc)


def kernel(x, W1, b1, W2, b2, question_types):
    global last_results
    x = np.ascontiguousarray(np.asarray(x, dtype=np.float32))
    W1 = np.asarray(W1, dtype=np.float32)
    b1 = np.asarray(b1, dtype=np.float32)
    W2 = np.asarray(W2, dtype=np.float32)
    b2 = np.asarray(b2, dtype=np.float32)
    qt = np.asarray(question_types)
    N = x.shape[0]

    idx = [np.nonzero(qt == e)[0] for e in range(E)]
    counts = [len(i) for i in idx]
    S = max(int(np.ceil(max(counts) / 16) * 16), 2 * CHUNK)

    nc = _program_cache.get(S)
    if nc is None:
        nc = _build_program(S)
        _program_cache[S] = nc

    mmnp = {"f32r": np.float32, "bf16": ml_dtypes.bfloat16}.get(MM_DTYPE, np.float32)
    in_maps = []
    for e in range(E):
        cnt = counts[e]
        xp = np.zeros((S, D), mmnp)
        xp[:cnt] = x[idx[e]].astype(mmnp)
        xt = np.ascontiguousarray(xp.T.reshape(DBLK, P, S).transpose(1, 0, 2))
        w1t = W1[e].reshape(DBLK, P, H).transpose(1, 0, 2).reshape(P, DBLK * H)
        w2t = W2[e].reshape(HBLK, P, C).transpose(1, 0, 2).reshape(P, HBLK * C)
        wt = np.ascontiguousarray(np.concatenate([w1t, w2t], axis=1)).astype(mmnp)
        bt = np.zeros((P, HBLK + 1), np.float32)
        bt[:, :HBLK] = b1[e].reshape(HBLK, P).T
        bt[:C, HBLK] = b2[e]
        in_maps.append({"xt": xt, "wt": wt, "bt": bt})

    r = run_bass_kernel_spmd(nc, in_maps, list(range(NCORES)))
    last_results = r

    out = np.zeros((N, C), np.float32)
    for e in range(E):
        out[idx[e]] = r.results[e]["yt"][:, : counts[e]].T
    return out



# revision 14
# speedup vs baseline: 1.0182x; 1.0031x over previous
"""Routed MoE classifier head for Trainium2 (8 NeuronCores, SPMD).

The reference computes all 8 experts densely and selects; here each sample is
routed to exactly one expert.  On the host we gather samples by expert
(expert e -> core e), pad to a common S, and pre-transpose x so the
contraction dim D lands on SBUF partitions.  Each core runs a dense 2-layer
MLP (768 -> relu 384 -> 8) over its expert's samples:

  layer 1:  h^T = relu(W1^T x^T + b1)   as matmul(psum, lhsT=W1 [128,128],
            rhs=xT [128,n]) accumulated over 6 d-blocks per h-block
  layer 2:  y^T = W2^T h^T + b2

Matmul operands use float32r (fp32 bits, 1 column/cycle streaming — 4x the
fp32 paired-pass rate — with ~11-mantissa-bit operand rounding); PSUM
accumulation stays fp32.  Output y^T [8, S] is scattered back on the host.
"""

import ml_dtypes
import numpy as np

import concourse.bass as bass
import concourse.mybir as mybir
from concourse.tile import TileContext
from concourse.bass_utils import run_bass_kernel_spmd

P = 128
D = 768
H = 384
C = 8
E = 8
NCORES = 8
DBLK = D // P  # 6
HBLK = H // P  # 3
CHUNK = 512  # compute chunk (one PSUM bank of fp32)
XGRAN = 1536  # x DMA granularity (samples per load; multiple of CHUNK)
YGRAN = 2048  # y DMA granularity (samples per store)

MM_DTYPE = "bf16"

_program_cache = {}
last_results = None  # BassKernelResults of the most recent run (for test harness)


def _split_excess_waits(nc, max_waits=1):
    """The walrus build in this container only encodes one sem-wait per
    instruction; hoist extra waits onto NOPs inserted just before."""
    for blk in nc.main_func.blocks:
        insts = blk.instructions
        i = 0
        while i < len(insts):
            inst = insts[i]
            si = getattr(inst, "sync_info", None)
            if si is not None and si.on_wait and len(si.on_wait) > max_waits:
                waits = list(si.on_wait)
                extra, keep = waits[:-max_waits], waits[-max_waits:]
                nops = []
                for j in range(0, len(extra), max_waits):
                    nops.append(
                        mybir.InstNoOp(
                            name=f"{inst.name}-wsplit{j}",
                            engine=inst.engine,
                            bass_nofuse=True,
                            sync_info=mybir.SyncInfo(
                                on_wait=extra[j : j + max_waits], on_update=[]
                            ),
                        )
                    )
                inst.sync_info = mybir.SyncInfo(on_wait=keep, on_update=si.on_update)
                for k, nop in enumerate(nops):
                    nc.register_instruction(nop, overwrite=True)
                    insts.insert(i + k, nop)
                i += len(nops)
            i += 1
    return nc


def _spans2(total, lead, gran):
    """[(off, n), ...] covering `total`: leading spans from `lead`, then
    `gran`-sized spans (last one smaller)."""
    spans = []
    off = 0
    k = 0
    while off < total:
        n = min(lead[k] if k < len(lead) else gran, total - off)
        spans.append((off, n))
        off += n
        k += 1
    return spans


def _build_program(S):
    f32 = mybir.dt.float32
    fmm = {"f32r": mybir.dt.float32r, "bf16": mybir.dt.bfloat16}.get(MM_DTYPE, f32)
    relu = mybir.ActivationFunctionType.Relu
    add = mybir.AluOpType.add

    nc = bass.Bass(enable_partition_id=False)
    xt = nc.dram_tensor("xt", [P, DBLK, S], fmm, kind="ExternalInput")
    # w1 (6*384 cols) and w2 (3*8 cols) packed on the same 128 partitions
    wt = nc.dram_tensor("wt", [P, DBLK * H + HBLK * C], fmm, kind="ExternalInput")
    # b1 (3 cols, per h-block) and b2 (1 col, rows 0..7) packed
    bt = nc.dram_tensor("bt", [P, HBLK + 1], f32, kind="ExternalInput")
    yt = nc.dram_tensor("yt", [C, S], f32, kind="ExternalOutput")

    x_spans = _spans2(S, [CHUNK, CHUNK, CHUNK], XGRAN)

    with TileContext(nc) as tc:
        with (
            tc.tile_pool(name="const", bufs=1) as cpool,
            tc.tile_pool(name="xin", bufs=3) as xpool,
            tc.tile_pool(name="hbuf", bufs=3) as hpool,
            tc.tile_pool(name="yout", bufs=2) as ypool,
            tc.tile_pool(name="psum1", bufs=6, space="PSUM") as pp1,
            tc.tile_pool(name="psum2", bufs=2, space="PSUM") as pp2,
        ):
            # DMA descriptor generation costs the issuing engine ~0.6us of
            # queue time per dma_start, so steady-state x spans are ONE
            # descriptor each on the sync queue (which otherwise only does
            # y stores); the scalar queue stays clean for relus.  Startup:
            # x span 0 drips per-d-block on sync (chunk 0 is db-outer), W1
            # rides scalar as [db=0 | rest] so the first matmul only waits
            # for its own 96KB.
            b_t = cpool.tile([P, HBLK + 1], f32)
            w_t = cpool.tile([P, DBLK * H + HBLK * C], fmm)
            span_tiles = {}
            nxt = 0  # next span index to issue

            def load_x_span(eng):
                nonlocal nxt
                si = nxt
                nxt += 1
                off, n = x_spans[si]
                x_t = xpool.tile([P, DBLK, XGRAN], fmm, name="x_t")
                eng.dma_start(x_t[:, :, :n], xt[:, :, off : off + n])
                span_tiles[si] = x_t

            # Interleave W1 d-block pieces and x0 d-block pieces across both
            # HWDGE queues so chunk 0's (db-outer) dependencies land in
            # consumption order ~0.7us apart — the PE must never idle >1us
            # once started, or the HAM clock-gate warmup window resets.
            off0, n0 = x_spans[0]
            x0 = xpool.tile([P, DBLK, XGRAN], fmm, name="x_t")
            for db in range(DBLK):
                eng = nc.sync if db % 2 == 0 else nc.scalar
                eng.dma_start(w_t[:, db * H : (db + 1) * H], wt[:, db * H : (db + 1) * H])
                eng.dma_start(x0[:, db, :n0], xt[:, db, off0 : off0 + n0])
            span_tiles[0] = x0
            nxt = 1
            nc.scalar.dma_start(b_t[:], bt[:])
            nc.scalar.dma_start(w_t[:, DBLK * H :], wt[:, DBLK * H :])  # W2
            if len(x_spans) > 1:
                load_x_span(nc.scalar)  # span 1
            if len(x_spans) > 2:
                load_x_span(nc.scalar)  # span 2

            # Warm the ACT table during the startup DMA window so the
            # first real relu doesn't pay the ~1.5us table load.
            warm = cpool.tile([P, 1], f32)
            nc.any.memset(warm[:], 0.0)
            nc.scalar.activation(warm[:], warm[:], relu, bias=0.0)

            y_tile = None  # current [C, YGRAN] output staging tile
            y_base = 0

            def emit_l2(pend):
                # layer 2 for an already-relu'd chunk: y^T = W2^T h^T + b2
                nonlocal y_tile, y_base
                h_t, off, n = pend
                ps2 = pp2.tile([C, CHUNK], f32, name="ps2")
                for hb in range(HBLK):
                    nc.tensor.matmul(
                        ps2[:, :n],
                        w_t[:, DBLK * H + hb * C : DBLK * H + (hb + 1) * C],
                        h_t[:, hb, :n],
                        start=(hb == 0),
                        stop=(hb == HBLK - 1),
                    )
                if y_tile is None:
                    y_tile = ypool.tile([C, YGRAN], f32, name="y_t")
                    y_base = off
                lo = off - y_base
                nc.vector.tensor_scalar(
                    y_tile[:, lo : lo + n],
                    ps2[:, :n],
                    scalar1=b_t[:C, HBLK : HBLK + 1],
                    scalar2=None,
                    op0=add,
                )
                if lo + n + CHUNK > YGRAN or off + n >= S:
                    # y stores ride the scalar queue: sync carries the big x
                    # span loads, and the final store must not queue behind
                    # one (the kernel cannot end before it completes).
                    nc.scalar.dma_start(
                        yt[:, y_base : y_base + lo + n], y_tile[:, : lo + n]
                    )
                    y_tile = None

            chunks = []  # (span_idx, global_off, local_off, n)
            for si, (soff, sn) in enumerate(x_spans):
                for o in range(0, sn, CHUNK):
                    chunks.append((si, soff + o, o, min(CHUNK, sn - o)))

            # Chunk 0 runs db-outer so each arriving x d-block slice feeds
            # all three h-block accumulators immediately (DMA drip-feed).
            # Steady-state chunks run hb-outer/db-inner so relu(hb) overlaps
            # the next h-block's matmul stream within the same chunk, and
            # layer-2 of chunk k-1 (emitted at the end of chunk k) has a
            # full chunk of slack behind relu(k-1, h2).
            pending = None
            for ci, (si, goff, o, n) in enumerate(chunks):
                x_t = span_tiles[si]
                if o == 0 and ci > 0 and nxt < len(x_spans):
                    load_x_span(nc.sync)
                h_t = hpool.tile([P, HBLK, CHUNK], fmm, name="h_t")
                if ci == 0:
                    pss = [pp1.tile([P, CHUNK], f32, name="ps") for _ in range(HBLK)]
                    for db in range(DBLK):
                        for hb in range(HBLK):
                            nc.tensor.matmul(
                                pss[hb][:, :n],
                                w_t[:, db * H + hb * P : db * H + (hb + 1) * P],
                                x_t[:, db, o : o + n],
                                start=(db == 0),
                                stop=(db == DBLK - 1),
                            )
                    for hb in range(HBLK):
                        nc.scalar.activation(
                            h_t[:, hb, :n], pss[hb][:, :n], relu,
                            bias=b_t[:, hb : hb + 1],
                        )
                else:
                    for hb in range(HBLK):
                        ps = pp1.tile([P, CHUNK], f32, name="ps")
                        for db in range(DBLK):
                            nc.tensor.matmul(
                                ps[:, :n],
                                w_t[:, db * H + hb * P : db * H + (hb + 1) * P],
                                x_t[:, db, o : o + n],
                                start=(db == 0),
                                stop=(db == DBLK - 1),
                            )
                        nc.scalar.activation(
                            h_t[:, hb, :n], ps[:, :n], relu,
                            bias=b_t[:, hb : hb + 1],
                        )
                if pending is not None:
                    emit_l2(pending)
                if o + n >= x_spans[si][1]:
                    span_tiles.pop(si, None)
                pending = (h_t, goff, n)
            emit_l2(pending)

    return _split_excess_waits(nc)


def kernel(x, W1, b1, W2, b2, question_types):
    global last_results
    x = np.ascontiguousarray(np.asarray(x, dtype=np.float32))
    W1 = np.asarray(W1, dtype=np.float32)
    b1 = np.asarray(b1, dtype=np.float32)
    W2 = np.asarray(W2, dtype=np.float32)
    b2 = np.asarray(b2, dtype=np.float32)
    qt = np.asarray(question_types)
    N = x.shape[0]

    idx = [np.nonzero(qt == e)[0] for e in range(E)]
    counts = [len(i) for i in idx]
    S = max(int(np.ceil(max(counts) / 16) * 16), 2 * CHUNK)

    nc = _program_cache.get(S)
    if nc is None:
        nc = _build_program(S)
        _program_cache[S] = nc

    mmnp = {"f32r": np.float32, "bf16": ml_dtypes.bfloat16}.get(MM_DTYPE, np.float32)
    in_maps = []
    for e in range(E):
        cnt = counts[e]
        xp = np.zeros((S, D), mmnp)
        xp[:cnt] = x[idx[e]].astype(mmnp)
        xt = np.ascontiguousarray(xp.T.reshape(DBLK, P, S).transpose(1, 0, 2))
        w1t = W1[e].reshape(DBLK, P, H).transpose(1, 0, 2).reshape(P, DBLK * H)
        w2t = W2[e].reshape(HBLK, P, C).transpose(1, 0, 2).reshape(P, HBLK * C)
        wt = np.ascontiguousarray(np.concatenate([w1t, w2t], axis=1)).astype(mmnp)
        bt = np.zeros((P, HBLK + 1), np.float32)
        bt[:, :HBLK] = b1[e].reshape(HBLK, P).T
        bt[:C, HBLK] = b2[e]
        in_maps.append({"xt": xt, "wt": wt, "bt": bt})

    r = run_bass_kernel_spmd(nc, in_maps, list(range(NCORES)))
    last_results = r

    out = np.zeros((N, C), np.float32)
    for e in range(E):
        out[idx[e]] = r.results[e]["yt"][:, : counts[e]].T
    return out

